# revision 1
# baseline (speedup 1.0000x reference)
"""Multi-head causal self-attention on 8 TRN2 NeuronCores.

Problem: x[2,2048,1024], 16 heads x 64 dim, causal softmax attention + output
projection. Sharding: core c -> (batch b=c//4, head-group g=c%4 of 4 heads).
Each core computes its 4 heads' attention output contribution for its batch;
a ReduceScatter over each 4-core group sums the head-group partials and
shards rows; the host concatenates the row shards.

Device-side layout trick: everything runs in "transposed score" space:
  qT/kT [dh, seq] come straight out of the projection matmuls (x fed
  pre-transposed from host), ST[kv,q] = K @ Q^T needs no transposes, the
  softmax denominator rides a ones-column appended to V (so it accumulates in
  the same PSUM tile as z^T), and z^T is exactly the lhsT the output
  projection wants. Softmax skips max-subtraction (scores are O(1) for this
  data distribution, exp cannot overflow in f32).

Compute dtype bf16 (PSUM accumulation f32); biases: b_Q/b_K folded in on
device as per-partition bias adds, b_V/b_O folded in on host (b_V enters the
output as a constant row because softmax rows sum to 1).
"""

import numpy as np
import ml_dtypes

import concourse.mybir as mybir
import concourse.tile as tile
from concourse import bacc
from concourse.bass_utils import run_bass_kernel_spmd

F32 = mybir.dt.float32
BF16 = mybir.dt.bfloat16
BF = ml_dtypes.bfloat16

B, S, D, H, DH = 2, 2048, 1024, 16, 64
HG = 4            # heads per core
NC = 8            # cores
QB = 512          # q block (columns per attention pass)
NJ = S // QB      # 4 q blocks
NKC = D // 128    # 8 contraction chunks for projections
NST = S // 128    # 16 seq tiles (kv tiles)
SCALE = 1.0 / np.sqrt(DH)
RG = [[0, 1, 2, 3], [4, 5, 6, 7]]


def build_kernel():
    nc = bacc.Bacc("TRN2", target_bir_lowering=False, debug=False, num_devices=NC)

    xt_ext = nc.dram_tensor("xt", [128, NKC, S], BF16, kind="ExternalInput")
    wq_ext = nc.dram_tensor("wq", [128, NKC, HG * DH], BF16, kind="ExternalInput")
    wk_ext = nc.dram_tensor("wk", [128, NKC, HG * DH], BF16, kind="ExternalInput")
    wv_ext = nc.dram_tensor("wv", [128, NKC, HG * DH], BF16, kind="ExternalInput")
    wo_ext = nc.dram_tensor("wo", [128, 2, D], BF16, kind="ExternalInput")
    bq_ext = nc.dram_tensor("bq", [128, 2], F32, kind="ExternalInput")
    bk_ext = nc.dram_tensor("bk", [128, 2], F32, kind="ExternalInput")
    out_ext = nc.dram_tensor("out", [NJ, 128, D], BF16, kind="ExternalOutput")

    Id = mybir.ActivationFunctionType.Identity
    Cp = mybir.ActivationFunctionType.Copy
    Exp = mybir.ActivationFunctionType.Exp

    with tile.TileContext(nc) as tc:
        with (
            tc.tile_pool(name="persist", bufs=1) as pp,
            tc.tile_pool(name="vaugp", bufs=1) as vp,
            tc.tile_pool(name="ptp", bufs=8) as ptp,
            tc.tile_pool(name="ztnp", bufs=8) as ztnp,
            tc.tile_pool(name="recipp", bufs=3) as recipp,
            tc.tile_pool(name="outsbp", bufs=8) as outsbp,
            tc.tile_pool(name="dram", bufs=NJ, space="DRAM") as dramp,
        ):
            # ---- persistent SBUF tensors ----
            xt_sb = pp.tile([128, NKC * S], BF16, name="xt_sb")
            wq_sb = pp.tile([128, NKC * HG * DH], BF16, name="wq_sb")
            wk_sb = pp.tile([128, NKC * HG * DH], BF16, name="wk_sb")
            wv_sb = pp.tile([128, NKC * HG * DH], BF16, name="wv_sb")
            wo_sb = pp.tile([128, 2 * D], BF16, name="wo_sb")
            bq_sb = pp.tile([128, 2], F32, name="bq_sb")
            bk_sb = pp.tile([128, 2], F32, name="bk_sb")
            qt_sb = [pp.tile([128, S], BF16, name=f"qt{p}") for p in range(2)]
            kt_sb = [pp.tile([128, S], BF16, name=f"kt{p}") for p in range(2)]
            masks2 = [pp.tile([128, 2 * QB], BF16, name=f"mask2_{rr}") for rr in range(2)]
            ones_sb = pp.tile([128, DH], BF16, name="ones_sb")

            wq_sb_v = wq_sb.rearrange("p (c m) -> p c m", c=NKC)
            wk_sb_v = wk_sb.rearrange("p (c m) -> p c m", c=NKC)
            for c in range(NKC):
                nc.sync.dma_start(wq_sb_v[:, c, :], wq_ext[:, c, :])
                nc.sync.dma_start(wk_sb_v[:, c, :], wk_ext[:, c, :])
            nc.sync.dma_start(bq_sb[:], bq_ext[:])
            nc.sync.dma_start(bk_sb[:], bk_ext[:])
            xt_sb_v = xt_sb.rearrange("p (c s) -> p c s", c=NKC)
            for q4 in range(4):
                nc.sync.dma_start(
                    xt_sb_v[:, 0, QB * q4 : QB * (q4 + 1)],
                    xt_ext[:, 0, QB * q4 : QB * (q4 + 1)],
                )
            for c in range(1, NKC):
                nc.sync.dma_start(xt_sb_v[:, c, :], xt_ext[:, c, :])
            nc.sync.dma_start(wv_sb[:], wv_ext[:])
            nc.sync.dma_start(wo_sb[:], wo_ext[:])

            nc.vector.memset(ones_sb[:], 1.0)
            for rr in range(2):
                nc.gpsimd.memset(masks2[rr][:], 1.0)
                for half in range(2):
                    r = 2 * rr + half
                    # keep (1.0) where kv_local(p) + 128*r <= q_local(f)
                    nc.gpsimd.affine_select(
                        out=masks2[rr][:, QB * half : QB * (half + 1)],
                        in_=masks2[rr][:, QB * half : QB * (half + 1)],
                        compare_op=mybir.AluOpType.is_ge,
                        fill=0.0,
                        base=-128 * r,
                        pattern=[[1, QB]],
                        channel_multiplier=-1,
                    )

            xt_v = xt_sb.rearrange("p (c s) -> p c s", c=NKC)
            wq_v = wq_sb.rearrange("p (c m) -> p c m", c=NKC)
            wk_v = wk_sb.rearrange("p (c m) -> p c m", c=NKC)
            wv_v = wv_sb.rearrange("p (c m) -> p c m", c=NKC)
            wo_v = wo_sb.rearrange("p (pr n) -> p pr n", pr=2)

            # ---- staged projection + attention, one stage per q block ----
            vaug = []
            with (
                tc.tile_pool(name="stps", bufs=2, space="PSUM") as stps,
                tc.tile_pool(name="ztps", bufs=2, space="PSUM") as ztps,
                tc.tile_pool(name="outps", bufs=2, space="PSUM") as outps,
                nc.allow_low_precision(reason="bf16 softmax/attn compute"),
            ):
                for j in range(NJ):
                    # qT/kT projection for column chunk j (both head pairs)
                    for p in range(2):
                        for w_v, t_sb, b_sb in ((wq_v, qt_sb, bq_sb), (wk_v, kt_sb, bk_sb)):
                            ps = outps.tile([128, QB], F32, tag="op")
                            for c in range(NKC):
                                nc.tensor.matmul(
                                    ps[:],
                                    w_v[:, c, 128 * p : 128 * (p + 1)],
                                    xt_v[:, c, QB * j : QB * (j + 1)],
                                    start=(c == 0),
                                    stop=(c == NKC - 1),
                                )
                            nc.scalar.activation(
                                t_sb[p][:, QB * j : QB * (j + 1)], ps[:],
                                Id, bias=b_sb[:, p : p + 1], scale=1.0,
                            )
                    # v projection for seq tiles 4j .. 4j+3
                    for t in range(4 * j, 4 * j + 4):
                        vt = vp.tile([128, 2 * 193], BF16, name=f"vaug{t}")
                        vaug.append(vt)
                        ps = outps.tile([128, QB], F32, tag="op")
                        for c in range(NKC):
                            nc.tensor.matmul(
                                ps[:, 0 : HG * DH],
                                xt_v[:, c, 128 * t : 128 * (t + 1)],
                                wv_v[:, c, :],
                                start=(c == 0),
                                stop=(c == NKC - 1),
                            )
                        psv = ps[:, 0 : HG * DH].rearrange(
                            "p (pr par d) -> p pr par d", pr=2, par=2
                        )
                        vt_pr = vt.rearrange("p (pr e) -> p pr e", pr=2)
                        # even head block [0:65] = [v(64) | 1]
                        # odd head block [65:193] = [1] + [0]*63 + v(64)
                        nc.scalar.activation(vt_pr[:, :, 0:DH], psv[:, :, 0, :], Cp)
                        nc.scalar.activation(vt_pr[:, :, 129:193], psv[:, :, 1, :], Cp)
                        nc.vector.memset(vt_pr[:, :, DH : DH + 1], 1.0)
                        nc.vector.memset(vt_pr[:, :, 65:129], 0.0)
                        nc.vector.memset(vt_pr[:, :, 65:66], 1.0)

                    nkv = 4 * (j + 1)
                    ztn2s = []
                    for pr in range(2):
                        zt_e = ztps.tile([128, QB], F32, tag="zt")
                        zt_o = ztps.tile([128, QB], F32, tag="zt")
                        zouts = (zt_e[0:65, :], zt_o[:, :])
                        for dd in range(nkv // 2):
                            pt2s = []
                            for par in range(2):
                                off = 64 * par
                                st2 = stps.tile([128, 2 * QB], F32, tag="st2")
                                for half in range(2):
                                    i = 2 * dd + half
                                    # cols < 128*r are fully causal-masked; skip
                                    # them in the score matmul (stale PSUM is
                                    # finite and zeroed by the mask multiply)
                                    lo = max(0, 128 * (i - 4 * j))
                                    nc.tensor.matmul(
                                        st2[:, QB * half + lo : QB * (half + 1)],
                                        kt_sb[pr][off : off + 64, 128 * i : 128 * (i + 1)],
                                        qt_sb[pr][off : off + 64, QB * j + lo : QB * (j + 1)],
                                        start=True,
                                        stop=True,
                                    )
                                pt2 = ptp.tile([128, 2 * QB], BF16, tag="pt2")
                                rr = dd - 2 * j
                                if rr == 1:
                                    # cols [0:256] fully causal-masked: zero them,
                                    # exp+mask only the live region
                                    nc.vector.memset(pt2[:, 0:256], 0.0)
                                    nc.scalar.activation(
                                        pt2[:, 256:], st2[:, 256:], Exp, scale=SCALE
                                    )
                                    nc.vector.tensor_mul(
                                        pt2[:, 256:], pt2[:, 256:], masks2[1][:, 256:]
                                    )
                                else:
                                    nc.scalar.activation(pt2[:], st2[:], Exp, scale=SCALE)
                                    if rr == 0:
                                        nc.vector.tensor_mul(pt2[:], pt2[:], masks2[0][:])
                                pt2s.append(pt2)
                            for par in range(2):
                                for half in range(2):
                                    i = 2 * dd + half
                                    lo = 193 * pr + (0 if par == 0 else 65)
                                    hi = 193 * pr + (65 if par == 0 else 193)
                                    nc.tensor.matmul(
                                        zouts[par],
                                        vaug[i][:, lo:hi],
                                        pt2s[par][:, QB * half : QB * (half + 1)],
                                        start=(i == 0),
                                        stop=(i == nkv - 1),
                                    )
                        # even head: denom row 64 -> DVE recip -> K=1 matmul bcast
                        recip = recipp.tile([65, QB], BF16, tag="recip")
                        nc.vector.reciprocal(recip[64:65, :], zt_e[64:65, :])
                        bc2 = outps.tile([64, QB], F32, tag="op")
                        nc.tensor.matmul(
                            bc2[:, :], ones_sb[64:65, 0:64], recip[64:65, :],
                            start=True, stop=True,
                        )
                        zsb2 = ztnp.tile([128, QB], BF16, tag="zsb")
                        nc.scalar.copy(zsb2[0:64, :], zt_e[0:64, :])
                        ztn2 = ztnp.tile([128, QB], BF16, tag="ztn")
                        nc.vector.tensor_mul(ztn2[0:64, :], zsb2[0:64, :], bc2[:, :])
                        # odd head: denom row 0 -> sbuf -> fast recip -> gpsimd bcast
                        den_o = recipp.tile([1, QB], F32, tag="den_o")
                        nc.scalar.copy(den_o[:, :], zt_o[0:1, :])
                        rec_o = recipp.tile([1, QB], F32, tag="rec_o")
                        nc.vector.reciprocal_approx_fast(rec_o[:, :], den_o[:, :])
                        bco = recipp.tile([128, QB], F32, tag="bco")
                        nc.gpsimd.partition_broadcast(bco[:, :], rec_o[:, :])
                        nc.vector.tensor_mul(
                            ztn2[64:128, :], zt_o[64:128, :], bco[64:128, :]
                        )
                        ztn2s.append(ztn2)

                    # output projection for this q block: K=128 per head pair
                    rs_in = dramp.tile([QB, D], BF16, name=f"rs_in{j}")
                    for qs in range(QB // 128):
                        osb = outsbp.tile([128, D], BF16, tag="osb")
                        for e in range(2):
                            op = outps.tile([128, QB], F32, tag="op")
                            for pr in range(2):
                                nc.tensor.matmul(
                                    op[:],
                                    ztn2s[pr][:, 128 * qs : 128 * (qs + 1)],
                                    wo_v[:, pr, 512 * e : 512 * (e + 1)],
                                    start=(pr == 0),
                                    stop=(pr == 1),
                                )
                            nc.vector.tensor_copy(osb[:, 512 * e : 512 * (e + 1)], op[:])
                        nc.sync.dma_start(rs_in[128 * qs : 128 * (qs + 1), :], osb[:])

                    rs_out = dramp.tile([128, D], BF16, name=f"rs_out{j}")
                    nc.gpsimd.collective_compute(
                        "ReduceScatter",
                        mybir.AluOpType.add,
                        replica_groups=RG,
                        ins=[rs_in.opt()],
                        outs=[rs_out.opt()],
                    )
                    nc.gpsimd.dma_start(out_ext[j], rs_out[:])

    nc.compile()
    return nc


_NC_CACHE = None


def _get_nc():
    global _NC_CACHE
    if _NC_CACHE is None:
        _NC_CACHE = build_kernel()
    return _NC_CACHE


def make_in_maps(x, W_Q, b_Q, W_K, b_K, W_V, b_V, W_O):
    def wlayout(W, hs):
        w = W[hs].transpose(1, 0, 2).reshape(D, HG * DH)
        return np.ascontiguousarray(
            w.reshape(NKC, 128, HG * DH).transpose(1, 0, 2)
        ).astype(BF)

    in_maps = []
    for c in range(NC):
        b, g = divmod(c, 4)
        hs = slice(HG * g, HG * (g + 1))
        xt = x[b].T  # [D, S]
        xt = np.ascontiguousarray(xt.reshape(NKC, 128, S).transpose(1, 0, 2)).astype(BF)
        in_maps.append(
            {
                "xt": xt,
                "wq": wlayout(W_Q, hs),
                "wk": wlayout(W_K, hs),
                "wv": wlayout(W_V, hs),
                "wo": np.ascontiguousarray(
                    W_O[hs].reshape(2, 2, DH, D).transpose(1, 2, 0, 3).reshape(128, 2, D)
                ).astype(BF),
                "bq": np.ascontiguousarray(b_Q[hs].reshape(2, 128).T).astype(np.float32),
                "bk": np.ascontiguousarray(b_K[hs].reshape(2, 128).T).astype(np.float32),
            }
        )
    return in_maps


def assemble_out(results, b_V, W_O, b_O):
    out = np.empty((B, S, D), np.float32)
    for c in range(NC):
        b, r = divmod(c, 4)
        o = results[c]["out"]  # [NJ, 128, D]
        for j in range(NJ):
            out[b, QB * j + 128 * r : QB * j + 128 * (r + 1), :] = o[j].astype(np.float32)
    # b_V enters the output as a constant row (softmax rows sum to 1); b_O too.
    out += np.einsum("hk,hkd->d", np.asarray(b_V, np.float32), np.asarray(W_O, np.float32))
    out += np.asarray(b_O, np.float32)
    return out


def kernel(
    normalized_resid_pre,
    W_Q,
    b_Q,
    W_K,
    b_K,
    W_V,
    b_V,
    W_O,
    b_O,
    _trace=False,
):
    x = np.asarray(normalized_resid_pre, np.float32)
    W_Q = np.asarray(W_Q, np.float32)
    W_K = np.asarray(W_K, np.float32)
    W_V = np.asarray(W_V, np.float32)
    W_O = np.asarray(W_O, np.float32)
    b_Q = np.asarray(b_Q, np.float32)
    b_K = np.asarray(b_K, np.float32)
    b_V = np.asarray(b_V, np.float32)
    b_O = np.asarray(b_O, np.float32)

    nc = _get_nc()
    in_maps = make_in_maps(x, W_Q, b_Q, W_K, b_K, W_V, b_V, W_O)
    res = run_bass_kernel_spmd(nc, in_maps, core_ids=list(range(NC)), trace=_trace)
    out = assemble_out(res.results, b_V, W_O, b_O)
    if _trace:
        return out, res
    return out



# revision 20
# speedup vs baseline: 1.1142x; 1.1142x over previous
"""Multi-head causal self-attention on 8 TRN2 NeuronCores.

Problem: x[2,2048,1024], 16 heads x 64 dim, causal softmax attention + output
projection. Sharding: core c -> (batch b=c//4, head-group g=c%4 of 4 heads).
Each core computes its 4 heads' attention output contribution for its batch;
a ReduceScatter over each 4-core group sums the head-group partials and
shards rows; the host concatenates the row shards.

Device-side layout trick: everything runs in "transposed score" space:
  qT/kT [dh, seq] come straight out of the projection matmuls (x fed
  pre-transposed from host), ST[kv,q] = K @ Q^T needs no transposes, the
  softmax denominator rides a ones-column appended to V (so it accumulates in
  the same PSUM tile as z^T), and z^T is exactly the lhsT the output
  projection wants. Softmax skips max-subtraction (scores are O(1) for this
  data distribution, exp cannot overflow in f32).

Engine balance: exp on ScalarE is the per-core floor (~10.5M pattern
elements); psum drains, bias adds and normalization run on VectorE, and the
PE stream alternates row groups so score matmul pairs run concurrently.
Softmax normalization: the denominator row is cast to bf16, broadcast with a
K=1 matmul, reciprocated with the fast approx op (custom DVE ops only work
at partition base 0 - verified on HW), and multiplied into z.

Compute dtype bf16 (PSUM accumulation f32); biases: b_Q/b_K folded in on
device as per-partition bias adds, b_V/b_O folded in on host (b_V enters the
output as a constant row because softmax rows sum to 1).
"""

import numpy as np
import ml_dtypes

import concourse.mybir as mybir
import concourse.tile as tile
from concourse import bacc
from concourse.bass_utils import run_bass_kernel_spmd

F32 = mybir.dt.float32
BF16 = mybir.dt.bfloat16
BF = ml_dtypes.bfloat16

B, S, D, H, DH = 2, 2048, 1024, 16, 64
HG = 4            # heads per core
NC = 8            # cores
QB = 512          # q block (columns per attention pass)
NJ = S // QB      # 4 q blocks
NKC = D // 128    # 8 contraction chunks for projections
NST = S // 128    # 16 seq tiles (kv tiles)
SCALE = 1.0 / np.sqrt(DH)
RG = [[0, 1, 2, 3], [4, 5, 6, 7]]


def build_kernel():
    nc = bacc.Bacc("TRN2", target_bir_lowering=False, debug=False, num_devices=NC)

    xt_ext = nc.dram_tensor("xt", [128, NKC, S], BF16, kind="ExternalInput")
    wq_ext = nc.dram_tensor("wq", [128, NKC, HG * DH], BF16, kind="ExternalInput")
    wk_ext = nc.dram_tensor("wk", [128, NKC, HG * DH], BF16, kind="ExternalInput")
    wv_ext = nc.dram_tensor("wv", [128, NKC, HG * DH], BF16, kind="ExternalInput")
    wo_ext = nc.dram_tensor("wo", [128, 2, D], BF16, kind="ExternalInput")
    bq_ext = nc.dram_tensor("bq", [128, 2], F32, kind="ExternalInput")
    bk_ext = nc.dram_tensor("bk", [128, 2], F32, kind="ExternalInput")
    out_ext = nc.dram_tensor("out", [NJ, 128, D], BF16, kind="ExternalOutput")

    Cp = mybir.ActivationFunctionType.Copy
    Exp = mybir.ActivationFunctionType.Exp

    with tile.TileContext(nc) as tc:
        with (
            tc.tile_pool(name="persist", bufs=1) as pp,
            tc.tile_pool(name="vaugp", bufs=1) as vp,
            tc.tile_pool(name="ptp", bufs=8) as ptp,
            tc.tile_pool(name="ztnp", bufs=8) as ztnp,
            tc.tile_pool(name="recipp", bufs=4) as recipp,
            tc.tile_pool(name="outsbp", bufs=8) as outsbp,
            tc.tile_pool(name="dram", bufs=NJ, space="DRAM") as dramp,
        ):
            # ---- persistent SBUF tensors ----
            xt_sb = pp.tile([128, NKC * S], BF16, name="xt_sb")
            wq_sb = pp.tile([128, NKC * HG * DH], BF16, name="wq_sb")
            wk_sb = pp.tile([128, NKC * HG * DH], BF16, name="wk_sb")
            wv_sb = pp.tile([128, NKC * HG * DH], BF16, name="wv_sb")
            wo_sb = pp.tile([128, 2 * D], BF16, name="wo_sb")
            bq_sb = pp.tile([128, 2], F32, name="bq_sb")
            bk_sb = pp.tile([128, 2], F32, name="bk_sb")
            qt_sb = [pp.tile([128, S], BF16, name=f"qt{p}") for p in range(2)]
            kt_sb = [pp.tile([128, S], BF16, name=f"kt{p}") for p in range(2)]
            masks2 = [pp.tile([128, 2 * QB], BF16, name=f"mask2_{rr}") for rr in range(2)]
            ones_sb = pp.tile([128, DH], BF16, name="ones_sb")

            wq_sb_v = wq_sb.rearrange("p (c m) -> p c m", c=NKC)
            wk_sb_v = wk_sb.rearrange("p (c m) -> p c m", c=NKC)
            xt_sb_v = xt_sb.rearrange("p (c s) -> p c s", c=NKC)
            # load order = first-use order: q/k weights, then x columns for
            # q block 0 (all contraction chunks), then v/o weights, then the
            # remaining x column blocks.
            for c in range(NKC):
                nc.sync.dma_start(wq_sb_v[:, c, :], wq_ext[:, c, :])
                nc.sync.dma_start(wk_sb_v[:, c, :], wk_ext[:, c, :])
            nc.sync.dma_start(bq_sb[:], bq_ext[:])
            nc.sync.dma_start(bk_sb[:], bk_ext[:])
            for c in range(NKC):
                nc.sync.dma_start(xt_sb_v[:, c, 0:QB], xt_ext[:, c, 0:QB])
            nc.sync.dma_start(wv_sb[:], wv_ext[:])
            for c in range(NKC):
                nc.sync.dma_start(
                    xt_sb_v[:, c, QB : 2 * QB], xt_ext[:, c, QB : 2 * QB]
                )
            nc.sync.dma_start(wo_sb[:], wo_ext[:])
            for q4 in range(2, 4):
                for c in range(NKC):
                    nc.sync.dma_start(
                        xt_sb_v[:, c, QB * q4 : QB * (q4 + 1)],
                        xt_ext[:, c, QB * q4 : QB * (q4 + 1)],
                    )

            nc.vector.memset(ones_sb[:], 1.0)
            for rr in range(2):
                nc.gpsimd.memset(masks2[rr][:], 1.0)
                for half in range(2):
                    r = 2 * rr + half
                    # keep (1.0) where kv_local(p) + 128*r <= q_local(f)
                    nc.gpsimd.affine_select(
                        out=masks2[rr][:, QB * half : QB * (half + 1)],
                        in_=masks2[rr][:, QB * half : QB * (half + 1)],
                        compare_op=mybir.AluOpType.is_ge,
                        fill=0.0,
                        base=-128 * r,
                        pattern=[[1, QB]],
                        channel_multiplier=-1,
                    )

            xt_v = xt_sb.rearrange("p (c s) -> p c s", c=NKC)
            wq_v = wq_sb.rearrange("p (c m) -> p c m", c=NKC)
            wk_v = wk_sb.rearrange("p (c m) -> p c m", c=NKC)
            wv_v = wv_sb.rearrange("p (c m) -> p c m", c=NKC)
            wo_v = wo_sb.rearrange("p (pr n) -> p pr n", pr=2)

            # ---- staged projection + attention, one stage per q block ----
            vaug = []
            with (
                tc.tile_pool(name="stps", bufs=2, space="PSUM") as stps,
                tc.tile_pool(name="ztps", bufs=2, space="PSUM") as ztps,
                tc.tile_pool(name="outps", bufs=2, space="PSUM") as outps,
                nc.allow_low_precision(reason="bf16 softmax/attn compute"),
            ):
                for j in range(NJ):
                    # qT/kT projection for column chunk j (both head pairs)
                    for p in range(2):
                        for w_v, t_sb, b_sb in ((wq_v, qt_sb, bq_sb), (wk_v, kt_sb, bk_sb)):
                            ps = outps.tile([128, QB], F32, tag="op")
                            for c in range(NKC):
                                nc.tensor.matmul(
                                    ps[:],
                                    w_v[:, c, 128 * p : 128 * (p + 1)],
                                    xt_v[:, c, QB * j : QB * (j + 1)],
                                    start=(c == 0),
                                    stop=(c == NKC - 1),
                                )
                            nc.vector.tensor_scalar_add(
                                t_sb[p][:, QB * j : QB * (j + 1)], ps[:],
                                b_sb[:, p : p + 1],
                            )
                    # v projection for seq tiles 4j .. 4j+3
                    for t in range(4 * j, 4 * j + 4):
                        vt = vp.tile([128, 2 * 193], BF16, name=f"vaug{t}")
                        vaug.append(vt)
                        ps = outps.tile([128, QB], F32, tag="op")
                        for c in range(NKC):
                            nc.tensor.matmul(
                                ps[:, 0 : HG * DH],
                                xt_v[:, c, 128 * t : 128 * (t + 1)],
                                wv_v[:, c, :],
                                start=(c == 0),
                                stop=(c == NKC - 1),
                            )
                        psv = ps[:, 0 : HG * DH].rearrange(
                            "p (pr par d) -> p pr par d", pr=2, par=2
                        )
                        vt_pr = vt.rearrange("p (pr e) -> p pr e", pr=2)
                        # even head block [0:65] = [v(64) | 1]
                        # odd head block [65:193] = [1] + [0]*63 + v(64)
                        nc.vector.tensor_copy(vt_pr[:, :, 0:DH], psv[:, :, 0, :])
                        nc.scalar.activation(vt_pr[:, :, 129:193], psv[:, :, 1, :], Cp)
                        nc.vector.memset(vt_pr[:, :, DH : DH + 1], 1.0)
                        nc.vector.memset(vt_pr[:, :, 65:129], 0.0)
                        nc.vector.memset(vt_pr[:, :, 65:66], 1.0)

                    nkv = 4 * (j + 1)
                    ztn2s = []
                    for pr in range(2):
                        zt_e = ztps.tile([128, QB], F32, tag="zt")
                        zt_o = ztps.tile([128, QB], F32, tag="zt")
                        zouts = (zt_e[0:65, :], zt_o[:, :])
                        for dd in range(nkv // 2):
                            # score matmuls, emitted half-major so consecutive
                            # matmuls alternate PE row groups (par 0 <-> 1):
                            # the row-tiled pair runs concurrently and each
                            # LDWEIGHTS pulls ahead of the other group's MM.
                            st2s = [
                                stps.tile([128, 2 * QB], F32, tag="st2", name="st2")
                                for _ in range(2)
                            ]
                            for half in range(2):
                                i = 2 * dd + half
                                # cols < 128*r are fully causal-masked; skip
                                # them in the score matmul (stale PSUM is
                                # finite and zeroed by the mask multiply)
                                lo = max(0, 128 * (i - 4 * j))
                                for par in range(2):
                                    off = 64 * par
                                    nc.tensor.matmul(
                                        st2s[par][:, QB * half + lo : QB * (half + 1)],
                                        kt_sb[pr][off : off + 64, 128 * i : 128 * (i + 1)],
                                        qt_sb[pr][off : off + 64, QB * j + lo : QB * (j + 1)],
                                        start=True,
                                        stop=True,
                                    )
                            pt2s = []
                            for par in range(2):
                                st2 = st2s[par]
                                pt2 = ptp.tile([128, 2 * QB], BF16, tag="pt2")
                                rr = dd - 2 * j
                                if rr == 1:
                                    # cols [0:256] fully causal-masked: zero them,
                                    # exp+mask only the live region
                                    nc.vector.memset(pt2[:, 0:256], 0.0)
                                    nc.scalar.activation(
                                        pt2[:, 256:], st2[:, 256:], Exp, scale=SCALE
                                    )
                                    nc.vector.tensor_mul(
                                        pt2[:, 256:], pt2[:, 256:], masks2[1][:, 256:]
                                    )
                                else:
                                    nc.scalar.activation(pt2[:], st2[:], Exp, scale=SCALE)
                                    if rr == 0:
                                        nc.vector.tensor_mul(pt2[:], pt2[:], masks2[0][:])
                                pt2s.append(pt2)
                            for par in range(2):
                                for half in range(2):
                                    i = 2 * dd + half
                                    lo = 193 * pr + (0 if par == 0 else 65)
                                    hi = 193 * pr + (65 if par == 0 else 193)
                                    nc.tensor.matmul(
                                        zouts[par],
                                        vaug[i][:, lo:hi],
                                        pt2s[par][:, QB * half : QB * (half + 1)],
                                        start=(i == 0),
                                        stop=(i == nkv - 1),
                                    )
                        # drain z psum to SBUF fast (frees banks for next pr)
                        zsb = ztnp.tile([128, QB], BF16, tag="zsb")
                        nc.vector.tensor_copy(zsb[0:64, :], zt_e[0:64, :])
                        nc.scalar.activation(zsb[64:128, :], zt_o[64:128, :], Cp)
                        # both denominator rows -> bf16, K=1 MM broadcast into
                        # one psum tile (even -> partitions 0..63, odd ->
                        # 64..127 via the col quadrant), then ONE approx recip
                        # at base 0 (custom DVE ops only work at base 0).
                        den_b = recipp.tile([65, QB], BF16, tag="den_b")
                        nc.vector.tensor_copy(den_b[64:65, :], zt_e[64:65, :])
                        den_ob = recipp.tile([1, QB], BF16, tag="den_ob")
                        nc.vector.tensor_copy(den_ob[:, :], zt_o[0:1, :])
                        bcd = outps.tile([128, QB], F32, tag="op")
                        nc.tensor.matmul(
                            bcd[0:64, :], ones_sb[64:65, 0:64], den_b[64:65, :],
                            start=True, stop=True,
                        )
                        nc.tensor.matmul(
                            bcd[64:128, :], ones_sb[0:1, 0:64], den_ob[0:1, :],
                            start=True, stop=True,
                        )
                        rcp = recipp.tile([128, QB], F32, tag="rcp")
                        nc.vector.reciprocal_approx_fast(rcp[:, :], bcd[:, :])
                        ztn2 = ztnp.tile([128, QB], BF16, tag="ztn")
                        nc.vector.tensor_mul(ztn2[:, :], zsb[:, :], rcp[:, :])
                        ztn2s.append(ztn2)

                    # output projection for this q block: K=128 per head pair
                    rs_in = dramp.tile([QB, D], BF16, name=f"rs_in{j}")
                    for qs in range(QB // 128):
                        osb = outsbp.tile([128, D], BF16, tag="osb")
                        for e in range(2):
                            op = outps.tile([128, QB], F32, tag="op")
                            for pr in range(2):
                                nc.tensor.matmul(
                                    op[:],
                                    ztn2s[pr][:, 128 * qs : 128 * (qs + 1)],
                                    wo_v[:, pr, 512 * e : 512 * (e + 1)],
                                    start=(pr == 0),
                                    stop=(pr == 1),
                                )
                            if e == 0:
                                nc.vector.tensor_copy(osb[:, 0:512], op[:])
                            else:
                                nc.scalar.activation(osb[:, 512:1024], op[:], Cp)
                        nc.sync.dma_start(rs_in[128 * qs : 128 * (qs + 1), :], osb[:])

                    rs_out = dramp.tile([128, D], BF16, name=f"rs_out{j}")
                    nc.gpsimd.collective_compute(
                        "ReduceScatter",
                        mybir.AluOpType.add,
                        replica_groups=RG,
                        ins=[rs_in.opt()],
                        outs=[rs_out.opt()],
                    )
                    nc.gpsimd.dma_start(out_ext[j], rs_out[:])

    nc.compile()
    return nc


_NC_CACHE = None


def _get_nc():
    global _NC_CACHE
    if _NC_CACHE is None:
        _NC_CACHE = build_kernel()
    return _NC_CACHE


def make_in_maps(x, W_Q, b_Q, W_K, b_K, W_V, b_V, W_O):
    def wlayout(W, hs):
        w = W[hs].transpose(1, 0, 2).reshape(D, HG * DH)
        return np.ascontiguousarray(
            w.reshape(NKC, 128, HG * DH).transpose(1, 0, 2)
        ).astype(BF)

    in_maps = []
    for c in range(NC):
        b, g = divmod(c, 4)
        hs = slice(HG * g, HG * (g + 1))
        xt = x[b].T  # [D, S]
        xt = np.ascontiguousarray(xt.reshape(NKC, 128, S).transpose(1, 0, 2)).astype(BF)
        in_maps.append(
            {
                "xt": xt,
                "wq": wlayout(W_Q, hs),
                "wk": wlayout(W_K, hs),
                "wv": wlayout(W_V, hs),
                "wo": np.ascontiguousarray(
                    W_O[hs].reshape(2, 2, DH, D).transpose(1, 2, 0, 3).reshape(128, 2, D)
                ).astype(BF),
                "bq": np.ascontiguousarray(b_Q[hs].reshape(2, 128).T).astype(np.float32),
                "bk": np.ascontiguousarray(b_K[hs].reshape(2, 128).T).astype(np.float32),
            }
        )
    return in_maps


def assemble_out(results, b_V, W_O, b_O):
    out = np.empty((B, S, D), np.float32)
    for c in range(NC):
        b, r = divmod(c, 4)
        o = results[c]["out"]  # [NJ, 128, D]
        for j in range(NJ):
            out[b, QB * j + 128 * r : QB * j + 128 * (r + 1), :] = o[j].astype(np.float32)
    # b_V enters the output as a constant row (softmax rows sum to 1); b_O too.
    out += np.einsum("hk,hkd->d", np.asarray(b_V, np.float32), np.asarray(W_O, np.float32))
    out += np.asarray(b_O, np.float32)
    return out


def kernel(
    normalized_resid_pre,
    W_Q,
    b_Q,
    W_K,
    b_K,
    W_V,
    b_V,
    W_O,
    b_O,
    _trace=False,
):
    x = np.asarray(normalized_resid_pre, np.float32)
    W_Q = np.asarray(W_Q, np.float32)
    W_K = np.asarray(W_K, np.float32)
    W_V = np.asarray(W_V, np.float32)
    W_O = np.asarray(W_O, np.float32)
    b_Q = np.asarray(b_Q, np.float32)
    b_K = np.asarray(b_K, np.float32)
    b_V = np.asarray(b_V, np.float32)
    b_O = np.asarray(b_O, np.float32)

    nc = _get_nc()
    in_maps = make_in_maps(x, W_Q, b_Q, W_K, b_K, W_V, b_V, W_O)
    res = run_bass_kernel_spmd(nc, in_maps, core_ids=list(range(NC)), trace=_trace)
    out = assemble_out(res.results, b_V, W_O, b_O)
    if _trace:
        return out, res
    return out


# revision 23
# speedup vs baseline: 1.2950x; 1.1623x over previous
"""Multi-head causal self-attention on 8 TRN2 NeuronCores.

Problem: x[2,2048,1024], 16 heads x 64 dim, causal softmax attention + output
projection. Sharding: core c -> (batch b=c//4, head-group g=c%4 of 4 heads).
Each core computes its 4 heads' attention output contribution for its batch;
a ReduceScatter over each 4-core group sums the head-group partials and
shards rows; the host concatenates the row shards.

Device-side layout trick: everything runs in "transposed score" space:
  qT/kT [dh, seq] come straight out of the projection matmuls (x fed
  pre-transposed from host), ST[kv,q] = K @ Q^T needs no transposes, the
  softmax denominator rides a ones-column appended to V (so it accumulates in
  the same PSUM tile as z^T), and z^T is exactly the lhsT the output
  projection wants. Softmax skips max-subtraction (scores are O(1) for this
  data distribution, exp cannot overflow in f32).

Engine balance: exp on ScalarE is the per-core floor (~10.5M pattern
elements); psum drains, bias adds and normalization run on VectorE, and the
PE stream alternates row groups so score matmul pairs run concurrently.
Softmax normalization: the denominator row is cast to bf16, broadcast with a
K=1 matmul, reciprocated with the fast approx op (custom DVE ops only work
at partition base 0 - verified on HW), and multiplied into z.

Compute dtype bf16 (PSUM accumulation f32); biases: b_Q/b_K folded in on
device as per-partition bias adds, b_V/b_O folded in on host (b_V enters the
output as a constant row because softmax rows sum to 1).
"""

import numpy as np
import ml_dtypes

import concourse.mybir as mybir
import concourse.tile as tile
from concourse import bacc
from concourse.bass_utils import run_bass_kernel_spmd

F32 = mybir.dt.float32
BF16 = mybir.dt.bfloat16
BF = ml_dtypes.bfloat16

B, S, D, H, DH = 2, 2048, 1024, 16, 64
HG = 4            # heads per core
NC = 8            # cores
QB = 512          # q block (columns per attention pass)
NJ = S // QB      # 4 q blocks
NKC = D // 128    # 8 contraction chunks for projections
NST = S // 128    # 16 seq tiles (kv tiles)
SCALE = 1.0 / np.sqrt(DH)
RG = [[0, 1, 2, 3], [4, 5, 6, 7]]


def build_kernel():
    nc = bacc.Bacc("TRN2", target_bir_lowering=False, debug=False, num_devices=NC)

    xt_ext = nc.dram_tensor("xt", [128, NKC, S], BF16, kind="ExternalInput")
    wq_ext = nc.dram_tensor("wq", [128, NKC, HG * DH], BF16, kind="ExternalInput")
    wk_ext = nc.dram_tensor("wk", [128, NKC, HG * DH], BF16, kind="ExternalInput")
    wv_ext = nc.dram_tensor("wv", [128, NKC, HG * DH], BF16, kind="ExternalInput")
    wo_ext = nc.dram_tensor("wo", [128, 2, D], BF16, kind="ExternalInput")
    bq_ext = nc.dram_tensor("bq", [128, 2], F32, kind="ExternalInput")
    bk_ext = nc.dram_tensor("bk", [128, 2], F32, kind="ExternalInput")
    out_ext = nc.dram_tensor("out", [NJ, 128, D], BF16, kind="ExternalOutput")

    Cp = mybir.ActivationFunctionType.Copy
    Exp = mybir.ActivationFunctionType.Exp

    with tile.TileContext(nc) as tc:
        with (
            tc.tile_pool(name="persist", bufs=1) as pp,
            tc.tile_pool(name="vaugp", bufs=1) as vp,
            tc.tile_pool(name="ptp", bufs=8) as ptp,
            tc.tile_pool(name="ztnp", bufs=8) as ztnp,
            tc.tile_pool(name="recipp", bufs=4) as recipp,
            tc.tile_pool(name="outsbp", bufs=8) as outsbp,
            tc.tile_pool(name="dram", bufs=NJ, space="DRAM") as dramp,
        ):
            # ---- persistent SBUF tensors ----
            xt_sb = pp.tile([128, NKC * S], BF16, name="xt_sb")
            wq_sb = pp.tile([128, NKC * HG * DH], BF16, name="wq_sb")
            wk_sb = pp.tile([128, NKC * HG * DH], BF16, name="wk_sb")
            wv_sb = pp.tile([128, NKC * HG * DH], BF16, name="wv_sb")
            wo_sb = pp.tile([128, 2 * D], BF16, name="wo_sb")
            bq_sb = pp.tile([128, 2], F32, name="bq_sb")
            bk_sb = pp.tile([128, 2], F32, name="bk_sb")
            qt_sb = [pp.tile([128, S], BF16, name=f"qt{p}") for p in range(2)]
            kt_sb = [pp.tile([128, S], BF16, name=f"kt{p}") for p in range(2)]
            masks2 = [pp.tile([128, 2 * QB], BF16, name=f"mask2_{rr}") for rr in range(2)]
            ones_sb = pp.tile([128, DH], BF16, name="ones_sb")

            wq_sb_v = wq_sb.rearrange("p (c m) -> p c m", c=NKC)
            wk_sb_v = wk_sb.rearrange("p (c m) -> p c m", c=NKC)
            xt_sb_v = xt_sb.rearrange("p (c s) -> p c s", c=NKC)
            # load order = first-use order: q/k weights, then x columns for
            # q block 0 (all contraction chunks), then v/o weights, then the
            # remaining x column blocks.
            for c in range(NKC):
                nc.sync.dma_start(wq_sb_v[:, c, :], wq_ext[:, c, :])
                nc.sync.dma_start(wk_sb_v[:, c, :], wk_ext[:, c, :])
            nc.sync.dma_start(bq_sb[:], bq_ext[:])
            nc.sync.dma_start(bk_sb[:], bk_ext[:])
            for c in range(NKC):
                nc.sync.dma_start(xt_sb_v[:, c, 0:QB], xt_ext[:, c, 0:QB])
            nc.sync.dma_start(wv_sb[:], wv_ext[:])
            for c in range(NKC):
                nc.sync.dma_start(
                    xt_sb_v[:, c, QB : 2 * QB], xt_ext[:, c, QB : 2 * QB]
                )
            nc.sync.dma_start(wo_sb[:], wo_ext[:])
            for q4 in range(2, 4):
                for c in range(NKC):
                    nc.sync.dma_start(
                        xt_sb_v[:, c, QB * q4 : QB * (q4 + 1)],
                        xt_ext[:, c, QB * q4 : QB * (q4 + 1)],
                    )

            nc.vector.memset(ones_sb[:], 1.0)
            for rr in range(2):
                nc.gpsimd.memset(masks2[rr][:], 1.0)
                for half in range(2):
                    r = 2 * rr + half
                    # keep (1.0) where kv_local(p) + 128*r <= q_local(f)
                    nc.gpsimd.affine_select(
                        out=masks2[rr][:, QB * half : QB * (half + 1)],
                        in_=masks2[rr][:, QB * half : QB * (half + 1)],
                        compare_op=mybir.AluOpType.is_ge,
                        fill=0.0,
                        base=-128 * r,
                        pattern=[[1, QB]],
                        channel_multiplier=-1,
                    )

            xt_v = xt_sb.rearrange("p (c s) -> p c s", c=NKC)
            wq_v = wq_sb.rearrange("p (c m) -> p c m", c=NKC)
            wk_v = wk_sb.rearrange("p (c m) -> p c m", c=NKC)
            wv_v = wv_sb.rearrange("p (c m) -> p c m", c=NKC)
            wo_v = wo_sb.rearrange("p (pr n) -> p pr n", pr=2)

            # ---- staged projection + attention, one stage per q block ----
            vaug = []
            with (
                tc.tile_pool(name="stps", bufs=2, space="PSUM") as stps,
                tc.tile_pool(name="ztps", bufs=2, space="PSUM") as ztps,
                tc.tile_pool(name="outps", bufs=2, space="PSUM") as outps,
                nc.allow_low_precision(reason="bf16 softmax/attn compute"),
            ):

                def outproj_emit(jj, zz):
                    """Output projection + ReduceScatter for block jj, as a
                    generator of PE-sized chunks. Block jj's chunks are
                    interleaved into block jj+1's exp-bound attention loop so
                    the PE never idles long enough to re-throttle."""
                    rs_in = dramp.tile([QB, D], BF16, name=f"rs_in{jj}")
                    for qs in range(QB // 128):
                        osb = outsbp.tile([128, D], BF16, tag="osb")
                        for e in range(2):
                            op = outps.tile([128, QB], F32, tag="op")
                            for pr_ in range(2):
                                nc.tensor.matmul(
                                    op[:],
                                    zz[pr_][:, 128 * qs : 128 * (qs + 1)],
                                    wo_v[:, pr_, 512 * e : 512 * (e + 1)],
                                    start=(pr_ == 0),
                                    stop=(pr_ == 1),
                                )
                            nc.vector.tensor_copy(
                                osb[:, 512 * e : 512 * (e + 1)], op[:]
                            )
                            yield
                        nc.sync.dma_start(rs_in[128 * qs : 128 * (qs + 1), :], osb[:])
                    rs_out = dramp.tile([128, D], BF16, name=f"rs_out{jj}")
                    nc.gpsimd.collective_compute(
                        "ReduceScatter",
                        mybir.AluOpType.add,
                        replica_groups=RG,
                        ins=[rs_in.opt()],
                        outs=[rs_out.opt()],
                    )
                    nc.gpsimd.dma_start(out_ext[jj], rs_out[:])
                    yield

                pending = None
                for j in range(NJ):
                    # qT/kT projection for column chunk j (both head pairs)
                    for p in range(2):
                        for w_v, t_sb, b_sb in ((wq_v, qt_sb, bq_sb), (wk_v, kt_sb, bk_sb)):
                            ps = outps.tile([128, QB], F32, tag="op")
                            for c in range(NKC):
                                nc.tensor.matmul(
                                    ps[:],
                                    w_v[:, c, 128 * p : 128 * (p + 1)],
                                    xt_v[:, c, QB * j : QB * (j + 1)],
                                    start=(c == 0),
                                    stop=(c == NKC - 1),
                                )
                            nc.vector.tensor_scalar_add(
                                t_sb[p][:, QB * j : QB * (j + 1)], ps[:],
                                b_sb[:, p : p + 1],
                            )
                    # v projection for seq tiles 4j .. 4j+3
                    for t in range(4 * j, 4 * j + 4):
                        vt = vp.tile([128, 2 * 193], BF16, name=f"vaug{t}")
                        vaug.append(vt)
                        ps = outps.tile([128, QB], F32, tag="op")
                        for c in range(NKC):
                            nc.tensor.matmul(
                                ps[:, 0 : HG * DH],
                                xt_v[:, c, 128 * t : 128 * (t + 1)],
                                wv_v[:, c, :],
                                start=(c == 0),
                                stop=(c == NKC - 1),
                            )
                        psv = ps[:, 0 : HG * DH].rearrange(
                            "p (pr par d) -> p pr par d", pr=2, par=2
                        )
                        vt_pr = vt.rearrange("p (pr e) -> p pr e", pr=2)
                        # even head block [0:65] = [v(64) | 1]
                        # odd head block [65:193] = [1] + [0]*63 + v(64)
                        nc.vector.tensor_copy(vt_pr[:, :, 0:DH], psv[:, :, 0, :])
                        nc.scalar.activation(vt_pr[:, :, 129:193], psv[:, :, 1, :], Cp)
                        nc.vector.memset(vt_pr[:, :, DH : DH + 1], 1.0)
                        nc.vector.memset(vt_pr[:, :, 65:129], 0.0)
                        nc.vector.memset(vt_pr[:, :, 65:66], 1.0)

                    nkv = 4 * (j + 1)
                    ztn2s = []
                    for pr in range(2):
                        zt_e = ztps.tile([128, QB], F32, tag="zt")
                        zt_o = ztps.tile([128, QB], F32, tag="zt")
                        zouts = (zt_e[0:65, :], zt_o[:, :])
                        for dd in range(nkv // 2):
                            # one deferred out-projection chunk of block j-1
                            # (PE filler while ScalarE chews the exps)
                            if pending is not None:
                                next(pending, None)
                            # score matmuls, emitted half-major so consecutive
                            # matmuls alternate PE row groups (par 0 <-> 1):
                            # the row-tiled pair runs concurrently and each
                            # LDWEIGHTS pulls ahead of the other group's MM.
                            st2s = [
                                stps.tile([128, 2 * QB], F32, tag="st2", name="st2")
                                for _ in range(2)
                            ]
                            for half in range(2):
                                i = 2 * dd + half
                                # cols < 128*r are fully causal-masked; skip
                                # them in the score matmul (stale PSUM is
                                # finite and zeroed by the mask multiply)
                                lo = max(0, 128 * (i - 4 * j))
                                for par in range(2):
                                    off = 64 * par
                                    nc.tensor.matmul(
                                        st2s[par][:, QB * half + lo : QB * (half + 1)],
                                        kt_sb[pr][off : off + 64, 128 * i : 128 * (i + 1)],
                                        qt_sb[pr][off : off + 64, QB * j + lo : QB * (j + 1)],
                                        start=True,
                                        stop=True,
                                    )
                            pt2s = []
                            for par in range(2):
                                st2 = st2s[par]
                                pt2 = ptp.tile([128, 2 * QB], BF16, tag="pt2")
                                rr = dd - 2 * j
                                if rr == 1:
                                    # cols [0:256] fully causal-masked: zero them,
                                    # exp+mask only the live region
                                    nc.vector.memset(pt2[:, 0:256], 0.0)
                                    nc.scalar.activation(
                                        pt2[:, 256:], st2[:, 256:], Exp, scale=SCALE
                                    )
                                    nc.vector.tensor_mul(
                                        pt2[:, 256:], pt2[:, 256:], masks2[1][:, 256:]
                                    )
                                else:
                                    nc.scalar.activation(pt2[:], st2[:], Exp, scale=SCALE)
                                    if rr == 0:
                                        nc.vector.tensor_mul(pt2[:], pt2[:], masks2[0][:])
                                pt2s.append(pt2)
                            for par in range(2):
                                for half in range(2):
                                    i = 2 * dd + half
                                    lo = 193 * pr + (0 if par == 0 else 65)
                                    hi = 193 * pr + (65 if par == 0 else 193)
                                    nc.tensor.matmul(
                                        zouts[par],
                                        vaug[i][:, lo:hi],
                                        pt2s[par][:, QB * half : QB * (half + 1)],
                                        start=(i == 0),
                                        stop=(i == nkv - 1),
                                    )
                        # drain z psum to SBUF fast (frees banks for next pr)
                        zsb = ztnp.tile([128, QB], BF16, tag="zsb")
                        nc.vector.tensor_copy(zsb[0:64, :], zt_e[0:64, :])
                        nc.scalar.activation(zsb[64:128, :], zt_o[64:128, :], Cp)
                        # both denominator rows -> bf16, K=1 MM broadcast into
                        # one psum tile (even -> partitions 0..63, odd ->
                        # 64..127 via the col quadrant), then ONE approx recip
                        # at base 0 (custom DVE ops only work at base 0).
                        den_b = recipp.tile([65, QB], BF16, tag="den_b")
                        nc.vector.tensor_copy(den_b[64:65, :], zt_e[64:65, :])
                        den_ob = recipp.tile([1, QB], BF16, tag="den_ob")
                        nc.vector.tensor_copy(den_ob[:, :], zt_o[0:1, :])
                        bcd = outps.tile([128, QB], F32, tag="op")
                        nc.tensor.matmul(
                            bcd[0:64, :], ones_sb[64:65, 0:64], den_b[64:65, :],
                            start=True, stop=True,
                        )
                        nc.tensor.matmul(
                            bcd[64:128, :], ones_sb[0:1, 0:64], den_ob[0:1, :],
                            start=True, stop=True,
                        )
                        rcp = recipp.tile([128, QB], F32, tag="rcp")
                        nc.vector.reciprocal_approx_fast(rcp[:, :], bcd[:, :])
                        ztn2 = ztnp.tile([128, QB], BF16, tag="ztn")
                        nc.vector.tensor_mul(ztn2[:, :], zsb[:, :], rcp[:, :])
                        ztn2s.append(ztn2)

                    # flush any leftover chunks of block j-1, then defer this
                    # block's output projection into block j+1 (last block
                    # runs inline)
                    if pending is not None:
                        for _ in pending:
                            pass
                    if j < NJ - 1:
                        pending = outproj_emit(j, ztn2s)
                    else:
                        for _ in outproj_emit(j, ztn2s):
                            pass

    nc.compile()
    return nc


_NC_CACHE = None


def _get_nc():
    global _NC_CACHE
    if _NC_CACHE is None:
        _NC_CACHE = build_kernel()
    return _NC_CACHE


def make_in_maps(x, W_Q, b_Q, W_K, b_K, W_V, b_V, W_O):
    def wlayout(W, hs):
        w = W[hs].transpose(1, 0, 2).reshape(D, HG * DH)
        return np.ascontiguousarray(
            w.reshape(NKC, 128, HG * DH).transpose(1, 0, 2)
        ).astype(BF)

    in_maps = []
    for c in range(NC):
        b, g = divmod(c, 4)
        hs = slice(HG * g, HG * (g + 1))
        xt = x[b].T  # [D, S]
        xt = np.ascontiguousarray(xt.reshape(NKC, 128, S).transpose(1, 0, 2)).astype(BF)
        in_maps.append(
            {
                "xt": xt,
                "wq": wlayout(W_Q, hs),
                "wk": wlayout(W_K, hs),
                "wv": wlayout(W_V, hs),
                "wo": np.ascontiguousarray(
                    W_O[hs].reshape(2, 2, DH, D).transpose(1, 2, 0, 3).reshape(128, 2, D)
                ).astype(BF),
                "bq": np.ascontiguousarray(b_Q[hs].reshape(2, 128).T).astype(np.float32),
                "bk": np.ascontiguousarray(b_K[hs].reshape(2, 128).T).astype(np.float32),
            }
        )
    return in_maps


def assemble_out(results, b_V, W_O, b_O):
    out = np.empty((B, S, D), np.float32)
    for c in range(NC):
        b, r = divmod(c, 4)
        o = results[c]["out"]  # [NJ, 128, D]
        for j in range(NJ):
            out[b, QB * j + 128 * r : QB * j + 128 * (r + 1), :] = o[j].astype(np.float32)
    # b_V enters the output as a constant row (softmax rows sum to 1); b_O too.
    out += np.einsum("hk,hkd->d", np.asarray(b_V, np.float32), np.asarray(W_O, np.float32))
    out += np.asarray(b_O, np.float32)
    return out


def kernel(
    normalized_resid_pre,
    W_Q,
    b_Q,
    W_K,
    b_K,
    W_V,
    b_V,
    W_O,
    b_O,
    _trace=False,
):
    x = np.asarray(normalized_resid_pre, np.float32)
    W_Q = np.asarray(W_Q, np.float32)
    W_K = np.asarray(W_K, np.float32)
    W_V = np.asarray(W_V, np.float32)
    W_O = np.asarray(W_O, np.float32)
    b_Q = np.asarray(b_Q, np.float32)
    b_K = np.asarray(b_K, np.float32)
    b_V = np.asarray(b_V, np.float32)
    b_O = np.asarray(b_O, np.float32)

    nc = _get_nc()
    in_maps = make_in_maps(x, W_Q, b_Q, W_K, b_K, W_V, b_V, W_O)
    res = run_bass_kernel_spmd(nc, in_maps, core_ids=list(range(NC)), trace=_trace)
    out = assemble_out(res.results, b_V, W_O, b_O)
    if _trace:
        return out, res
    return out
